# revision 1
# baseline (speedup 1.0000x reference)
"""Trainium2 Bass kernel for nn_Attention (LN -> QKV -> softmax attn -> out proj).

Sharding: 8 cores; core c handles batch b=c//4 and heads [4*(c%4), 4*(c%4)+4).
Each core computes two partial output contributions (one per head-pair stack)
of shape [1024, 2048] = (w_out slice).T @ attn_out.T; the host sums the 8
partials per batch, transposes, and adds b_out.

Device pipeline per core (bf16 matmuls, fp32 PSUM accumulate):
  A) LN stats on the PE: Sx = 1^T x, Sxx = 1^T x^2 (x^2 via one ScalarE
     Square) accumulated into PSUM rows; rsqrt(var+eps) via bit-trick seed +
     3 Newton steps on the DVE (no ACT table switches, no slow reciprocal).
  B) QKV on raw x^T with LayerNorm folded in algebraically:
       qkv[t,c] = r[t] * ((x @ W')[t,c] - mu[t]*u[c]) + (beta @ W)[c]
     where W' = gamma-folded (and q-scaled) weights, u = colsum(W').
     The -mu*u term is a rank-1 K=1 matmul accumulated straight into each
     QKV PSUM group. q,k are produced transposed [dh, t] (k stays un-scaled:
     r_k rides the exp's per-partition scale); v comes out natural [t, dh].
  C) Flash-style attention without running max (matches the reference
     exactly: plain exp, denom = sum + eps which is a no-op in fp32): S^T
     tiles via matmul, exp on ScalarE (psum -> sbuf bf16), P@V via matmul
     with a ones-column appended to v so the denominator accumulates in the
     same PSUM tile; per-qp-half denominator reciprocal via DMA-reshape +
     Newton, broadcast back through DRAM.
  D) Output projection split into two independent partials, interleaved into
     the last head's attention loop to overlap the PE.
"""

import contextlib

import numpy as np

import concourse.bass as bass
import concourse.tile as tile
from concourse import bacc, mybir
from concourse import bass_utils

# Problem constants (hardcoded per contract)
B, N, DIM = 2, 2048, 1024
H, DH = 16, 64
INNER = H * DH
LN_EPS = 1e-5
ATTN_EPS = 1e-8
SCALE = DH ** -0.5

# Per-core constants
P = 128
T = N                 # tokens per core (one batch)
TT = T // P           # 16 token tiles of 128
NT4 = T // 512        # 4 token tiles of 512
KD = DIM // P         # 8 contraction tiles
HL = 4                # local heads per core
CQK = 2 * HL * DH     # 512 (q cols + k cols)
CV = HL * DH          # 256 (v cols)
GQK = CQK // P        # 4 col groups of 128
KT = T // P           # 16 key tiles of 128

f32 = mybir.dt.float32
f32r = mybir.dt.float32r
bf16 = mybir.dt.bfloat16
FT = mybir.ActivationFunctionType
ALU = mybir.AluOpType

import ml_dtypes
_BF16 = np.dtype(ml_dtypes.bfloat16)

_CACHE = {}


def _hrows(h):
    """Partition slice for head h within a [128, 2, T] two-stack layout."""
    lo = 64 * (h % 2)
    return slice(lo, lo + 64), h // 2


def _build(has_v0):
    nc = bacc.Bacc("TRN2", target_bir_lowering=False, debug=False)

    xt_d = nc.dram_tensor("xt", [DIM, T], bf16, kind="ExternalInput").ap()
    wqk_d = nc.dram_tensor("wqk", [DIM, CQK], bf16, kind="ExternalInput").ap()
    wv_d = nc.dram_tensor("wv", [DIM, CV], bf16, kind="ExternalInput").ap()
    wout_d = nc.dram_tensor("wout", [2 * P, DIM], f32r, kind="ExternalInput").ap()
    nuqk16_d = nc.dram_tensor("nuqk", [CQK], bf16, kind="ExternalInput").ap()
    nuv_d = nc.dram_tensor("nuv", [CV], bf16, kind="ExternalInput").ap()
    v0qk_d = nc.dram_tensor("v0qk", [CQK], f32, kind="ExternalInput").ap()
    v0v_d = nc.dram_tensor("v0v", [CV], f32, kind="ExternalInput").ap()
    outp0_d = nc.dram_tensor("outp0", [DIM, T], f32, kind="ExternalOutput").ap()
    outp1_d = nc.dram_tensor("outp1", [DIM, T], f32, kind="ExternalOutput").ap()

    with tile.TileContext(nc) as tc, contextlib.ExitStack() as ctx:
        pers = ctx.enter_context(tc.tile_pool(name="pers", bufs=1))
        dram = ctx.enter_context(tc.tile_pool(name="dram", bufs=1, space="DRAM"))

        qkT = pers.tile([P, GQK, T], bf16)          # q/k transposed, heads stacked
        vaug = pers.tile([P, KT, HL, DH + 1], bf16)  # v + ones column
        outT = pers.tile([P, 2, T], f32r)           # attention output (transposed)
        wout_sb = pers.tile([P, 2, DIM], f32r)
        dnm = [pers.tile([1, T], f32, name=f"dnm{i}") for i in range(HL)]
        r_c = pers.tile([P, TT], f32)

        sx_dram = dram.tile([1, T], f32)
        sxx_dram = dram.tile([1, T], f32)
        r_dram = dram.tile([1, T], f32)
        dnm_dram = dram.tile([HL, T], f32)
        rdn_dram = dram.tile([HL, T], f32)

        nc.vector.memset(vaug[:], 1.0)

        # ---------------- Phase A+B: stats + QKV projection ----------------
        with tc.tile_pool(name="pab", bufs=1) as pab, \
             tc.tile_pool(name="pabd", bufs=4) as pabd, \
             tc.tile_pool(name="pgen", bufs=4, space="PSUM") as pgen, \
             tc.tile_pool(name="pgv", bufs=3, space="PSUM") as pgv, \
             tc.tile_pool(name="pgs", bufs=1, space="PSUM") as pgs:

            wqk_sb = pab.tile([P, KD, CQK], bf16)
            nc.sync.dma_start(wqk_sb[:], wqk_d.rearrange("(o p) c -> p o c", p=P))
            wv_sb = pab.tile([P, KD, CV], bf16)
            nc.sync.dma_start(wv_sb[:], wv_d.rearrange("(o p) c -> p o c", p=P))
            uqkr_sb = pab.tile([1, CQK], bf16)
            nc.sync.dma_start(uqkr_sb[:], nuqk16_d[None, :])
            uvr_sb = pab.tile([1, CV], bf16)
            nc.sync.dma_start(uvr_sb[:], nuv_d[None, :])
            nmu_row = pab.tile([1, T], bf16)
            sx_row = pab.tile([1, T], f32)
            sxx_row = pab.tile([1, T], f32)
            ones_col = pab.tile([P, 1], bf16)
            nc.vector.memset(ones_col[:], 1.0)
            if has_v0:
                v0qk_sb = pab.tile([P, GQK], f32)
                nc.sync.dma_start(v0qk_sb[:], v0qk_d.rearrange("(g p) -> p g", p=P))
                v0v_bc = pab.tile([P, CV], f32)
                nc.sync.dma_start(v0v_bc[:], v0v_d[None, :].to_broadcast([P, CV]))

            r_bc = pab.tile([P, T], f32)
            sxc = pab.tile([P, TT], f32)
            sxxc = pab.tile([P, TT], f32)
            mu_cc = pab.tile([P, TT], f32)
            magic = pab.tile([P, TT], mybir.dt.int32)
            nc.vector.memset(magic[:], 0x5F3759DF)
            ex2e = pab.tile([P, TT], f32)
            mu2 = pab.tile([P, TT], f32)
            ve = pab.tile([P, TT], f32)
            y0i = pab.tile([P, TT], mybir.dt.int32)
            t0 = pab.tile([P, TT], f32)

            bones = pab.tile([1, 1], bf16)
            nc.vector.memset(bones[:], 1.0)
            brow = pab.tile([1, 64], bf16)
            nc.vector.memset(brow[:], 1.0)
            warm_ps = pgs.tile([P, 512], f32, tag="st", name="warm0")
            for _ in range(150):
                nc.tensor.matmul(warm_ps[64:65, 0:64], bones[:], brow[:],
                                 start=True, stop=True)

            def load_xt(t4):
                tsl = slice(t4 * 512, (t4 + 1) * 512)
                xt_t = pabd.tile([P, KD, 512], bf16, tag="xt", name=f"xt{t4}")
                for kt in range(KD):
                    nc.sync.dma_start(
                        xt_t[:, kt],
                        xt_d[kt * P:(kt + 1) * P, tsl])
                return xt_t

            xt_tiles = {0: load_xt(0)}

            for t4 in range(NT4):
                tsl = slice(t4 * 512, (t4 + 1) * 512)
                s4 = slice(t4 * 4, t4 * 4 + 4)
                if t4 + 1 < NT4:
                    xt_tiles[t4 + 1] = load_xt(t4 + 1)

                # --- stats on PE: Sx = 1^T x, Sxx = 1^T x^2 (psum rows) ---
                xt_t = xt_tiles.pop(t4)
                xsq = pabd.tile([P, KD, 512], bf16, tag="xsq")
                nc.scalar.activation(xsq[:], xt_t[:], FT.Square)
                ps_st = pgs.tile([P, 512], f32, tag="st", name=f"st{t4}")
                for kt in range(KD):
                    nc.tensor.matmul(ps_st[0:1], ones_col[:], xt_t[:, kt],
                                     start=(kt == 0), stop=(kt == KD - 1))
                for kt in range(KD):
                    nc.tensor.matmul(ps_st[32:33], ones_col[:], xsq[:, kt],
                                     start=(kt == 0), stop=(kt == KD - 1))
                nc.scalar.activation(nmu_row[0:1, tsl], ps_st[0:1], FT.Copy,
                                     scale=-1.0 / DIM)
                nc.scalar.copy(sx_row[0:1, tsl], ps_st[0:1])
                nc.scalar.copy(sxx_row[0:1, tsl], ps_st[32:33])
                # round-trip rows into [128, 4] column layout for rsqrt
                nc.sync.dma_start(sx_dram[0:1, tsl], sx_row[0:1, tsl])
                nc.sync.dma_start(sxx_dram[0:1, tsl], sxx_row[0:1, tsl])
                nc.sync.dma_start(sxc[:, s4],
                                  sx_dram[0, tsl].rearrange("(o p) -> p o", p=P))
                nc.sync.dma_start(sxxc[:, s4],
                                  sxx_dram[0, tsl].rearrange("(o p) -> p o", p=P))

                # --- finalize r = rsqrt(var + eps) in column layout ---
                nc.vector.tensor_scalar(ex2e[:, s4], sxxc[:, s4], 1.0 / DIM,
                                        LN_EPS, ALU.mult, ALU.add)
                nc.vector.tensor_scalar_mul(mu_cc[:, s4], sxc[:, s4], 1.0 / DIM)
                nc.vector.tensor_tensor(mu2[:, s4], mu_cc[:, s4], mu_cc[:, s4],
                                        ALU.mult)
                nc.vector.scalar_tensor_tensor(ve[:, s4], mu2[:, s4], -1.0,
                                               ex2e[:, s4], ALU.mult, ALU.add)
                nc.vector.tensor_scalar(y0i[:, s4],
                                        ve[:, s4].bitcast(mybir.dt.int32), 1,
                                        None, ALU.arith_shift_right)
                nc.vector.tensor_tensor(y0i[:, s4], magic[:, s4], y0i[:, s4],
                                        ALU.subtract)
                y = y0i.bitcast(f32)
                for _ in range(3):
                    nc.vector.tensor_tensor(t0[:, s4], y[:, s4], y[:, s4],
                                            ALU.mult)
                    nc.vector.tensor_tensor(t0[:, s4], t0[:, s4], ve[:, s4],
                                            ALU.mult)
                    nc.vector.tensor_scalar(t0[:, s4], t0[:, s4], -0.5, 1.5,
                                            ALU.mult, ALU.add)
                    nc.vector.tensor_tensor(y[:, s4], y[:, s4], t0[:, s4],
                                            ALU.mult)
                nc.vector.tensor_copy(r_c[:, s4], y[:, s4])

                # stats slab to DRAM, then broadcast across partitions
                nc.sync.dma_start(
                    r_dram[0, tsl].rearrange("(o p) -> p o", p=P), r_c[:, s4])
                nc.sync.dma_start(r_bc[:, tsl],
                                  r_dram[0:1, tsl].to_broadcast([P, 512]))

                # --- QKV matmuls + LN-fold corrections for this slab ---
                pending = []

                def finish_qk(nc=nc, tsl=tsl):
                    g, ps = pending.pop(0)
                    # rank-1 LayerNorm-mean correction: psum += u * (-mu)^T
                    nc.tensor.matmul(ps[:], uqkr_sb[0:1, g * P:(g + 1) * P],
                                     nmu_row[0:1, tsl], start=False, stop=True)
                    # plain copy frees the psum slot; r_q applied in a
                    # deferred in-place pass below (r_k rides the exp scale)
                    if g >= 2:
                        nc.scalar.copy(qkT[:, g, tsl], ps[:])
                    else:
                        nc.vector.tensor_copy(qkT[:, g, tsl], ps[:])
                    if has_v0:
                        nc.vector.tensor_scalar_add(qkT[:, g, tsl],
                                                    qkT[:, g, tsl],
                                                    v0qk_sb[:, g:g + 1])

                for g in (2, 3, 0, 1):      # k groups first: no r_bc dep
                    ps = pgen.tile([P, 512], f32, tag="qk", name=f"qk{t4}_{g}")
                    for kt in range(KD):
                        nc.tensor.matmul(ps[:], wqk_sb[:, kt, g * P:(g + 1) * P],
                                         xt_t[:, kt],
                                         start=(kt == 0), stop=False)
                    pending.append((g, ps))
                    if len(pending) > 1:
                        finish_qk()
                while pending:
                    finish_qk()
                # deferred r_q multiply (in place, both q groups)
                nc.vector.tensor_tensor(
                    qkT[:, 0:2, tsl], qkT[:, 0:2, tsl],
                    r_bc[:, tsl][:, None, :].to_broadcast([P, 2, 512]),
                    ALU.mult)

                for st in range(4):
                    tts = t4 * 4 + st
                    stsl = slice(t4 * 512 + st * P, t4 * 512 + (st + 1) * P)
                    psv = pgv.tile([P, CV], f32, tag="v", name=f"v{t4}_{st}")
                    for kt in range(KD):
                        nc.tensor.matmul(psv[:],
                                         xt_t[:, kt, st * P:(st + 1) * P],
                                         wv_sb[:, kt],
                                         start=(kt == 0), stop=False)
                    nc.tensor.matmul(psv[:], nmu_row[0:1, stsl], uvr_sb[0:1, :],
                                     start=False, stop=True)
                    psv3 = psv.rearrange("p (h d) -> p h d", h=HL)
                    nc.vector.tensor_scalar_mul(vaug[:, tts, :, 0:DH], psv3,
                                                r_c[:, tts:tts + 1])
                    if has_v0:
                        v03 = v0v_bc.rearrange("p (h d) -> p h d", h=HL)
                        nc.vector.tensor_tensor(vaug[:, tts, :, 0:DH],
                                                vaug[:, tts, :, 0:DH],
                                                v03, ALU.add)

        # ---------------- Phase C: attention (+ overlapped out-proj) -------
        with tc.tile_pool(name="pat", bufs=6) as pat, \
             tc.tile_pool(name="pat1", bufs=1) as pat1, \
             tc.tile_pool(name="pdo", bufs=3) as pdo, \
             tc.tile_pool(name="psc", bufs=2, space="PSUM") as psc, \
             tc.tile_pool(name="ppv", bufs=2, space="PSUM") as ppv, \
             tc.tile_pool(name="pop", bufs=2, space="PSUM") as pop:

            dbc = pat1.tile([P, 2, T], f32)
            nc.sync.dma_start(wout_sb[:], wout_d.rearrange("(o p) c -> p o c", p=P))
            wones = pat1.tile([1, 1], bf16)
            nc.vector.memset(wones[:], 1.0)
            wrow = pat1.tile([1, 64], bf16)
            nc.vector.memset(wrow[:], 1.0)

            def keep_warm(n, ps_ap):
                for i in range(n):
                    nc.tensor.matmul(ps_ap, wones[:], wrow[:],
                                     start=True, stop=True)

            def outproj_unit(stk2, od, oc, t4):
                tsl = slice(t4 * 512, (t4 + 1) * 512)
                ps = pop.tile([P, 512], f32, tag="op",
                              name=f"op{stk2}_{oc}_{t4}")
                nc.tensor.matmul(ps[:],
                                 wout_sb[:, stk2, oc * P:(oc + 1) * P],
                                 outT[:, stk2, tsl], start=True, stop=True)
                osb = pdo.tile([P, 512], f32, tag="osb")
                nc.any.tensor_copy(osb[:], ps[:])
                nc.sync.dma_start(od[oc * P:(oc + 1) * P, tsl], osb[:])

            op0_units = [(oc, t4) for oc in range(DIM // P)
                         for t4 in range(NT4)]
            op1_units = [(oc, t4) for t4 in range(NT4)
                         for oc in range(DIM // P)]

            for h in range(HL):
                rows, stk = _hrows(h)
                if h == 0:
                    wsc = psc.tile([P, 2, 512], f32, tag="sc", name="warm_bc")
                    keep_warm(48, wsc[0:1, 0, 0:64])
                for qp in range(2):
                    ps_o = [ppv.tile([DH + 1, 512], f32, tag="pv",
                                     name=f"pv{h}_{qp}_{i}") for i in range(2)]
                    for kt in range(KT):
                        if h == 3 and qp == 0 and op0_units:
                            oc_, t4_ = op0_units.pop(0)
                            outproj_unit(0, outp0_d, oc_, t4_)
                        if h == 3 and qp == 1:
                            if op0_units:
                                oc_, t4_ = op0_units.pop(0)
                                outproj_unit(0, outp0_d, oc_, t4_)
                            if op1_units and op1_units[0][1] < 2 and kt >= 8:
                                oc_, t4_ = op1_units.pop(0)
                                outproj_unit(1, outp1_d, oc_, t4_)
                        ps_s = psc.tile([P, 2, 512], f32, tag="sc",
                                        name=f"sc{h}_{qp}_{kt}")
                        for sub in range(2):
                            qt = qp * 2 + sub
                            nc.tensor.matmul(
                                ps_s[:, sub],
                                qkT[rows, 2 + stk, kt * P:(kt + 1) * P],
                                qkT[rows, stk, qt * 512:(qt + 1) * 512],
                                start=True, stop=True)
                        et = pat.tile([P, 2, 512], bf16, tag="exp",
                                      name=f"et{h}_{qp}_{kt}")
                        nc.scalar.activation(et[:], ps_s[:], FT.Exp,
                                             scale=r_c[:, kt:kt + 1])
                        for sub in range(2):
                            nc.tensor.matmul(ps_o[sub], vaug[:, kt, h, :],
                                             et[:, sub],
                                             start=(kt == 0), stop=(kt == KT - 1))
                    for sub in range(2):
                        qt = qp * 2 + sub
                        qsl = slice(qt * 512, (qt + 1) * 512)
                        nc.vector.tensor_copy(dnm[h][0:1, qsl],
                                              ps_o[sub][DH:DH + 1])
                        nc.vector.tensor_copy(outT[rows, stk, qsl],
                                              ps_o[sub][0:DH])

                    # per-qp-half denominator reciprocal + normalize:
                    # DMA-reshape to [128, 8], bit-trick seed + 3 Newton steps
                    hsl = slice(qp * 1024, (qp + 1) * 1024)
                    nc.sync.dma_start(dnm_dram[h:h + 1, hsl], dnm[h][0:1, hsl])
                    dn2 = pat.tile([P, TT // 2], f32, tag="dn2")
                    nc.sync.dma_start(
                        dn2[:], dnm_dram[h, hsl].rearrange("(p o) -> p o", p=P))
                    rmagic = pat.tile([P, TT // 2], mybir.dt.int32, tag="rmagic")
                    nc.vector.memset(rmagic[:], 0x7EEF362E)
                    yi = pat.tile([P, TT // 2], mybir.dt.int32, tag="yi")
                    nc.vector.tensor_tensor(yi[:], rmagic[:],
                                            dn2[:].bitcast(mybir.dt.int32),
                                            ALU.subtract)
                    yf = yi.bitcast(f32)
                    tn = pat.tile([P, TT // 2], f32, tag="tn")
                    for _ in range(3):
                        nc.vector.tensor_tensor(tn[:], dn2[:], yf[:], ALU.mult)
                        nc.vector.tensor_scalar(tn[:], tn[:], -1.0, 2.0,
                                                ALU.mult, ALU.add)
                        nc.vector.tensor_tensor(yf[:], yf[:], tn[:], ALU.mult)
                    nc.sync.dma_start(
                        rdn_dram[h, hsl].rearrange("(p o) -> p o", p=P), yf[:])
                    nc.sync.dma_start(
                        dbc[rows, stk, hsl],
                        rdn_dram[h:h + 1, hsl].to_broadcast([64, 1024]))
                    nc.vector.tensor_tensor(outT[rows, stk, hsl],
                                            outT[rows, stk, hsl],
                                            dbc[rows, stk, hsl], ALU.mult)
                    # interleave stack-1 out-proj for ready halves during h3
                    if h == 3 and qp == 1 and op1_units:
                        while op1_units and op1_units[0][1] < 2:
                            oc_, t4_ = op1_units.pop(0)
                            outproj_unit(1, outp1_d, oc_, t4_)

            # ------------ Phase D: remaining output projection ----------
            for oc_, t4_ in op0_units:
                outproj_unit(0, outp0_d, oc_, t4_)
            for oc_, t4_ in op1_units:
                outproj_unit(1, outp1_d, oc_, t4_)

    nc.compile()
    return nc


def _prep_inputs(x, ln_gamma, ln_beta, w_qkv, w_out, b_out):
    """Host-side sharding/layout prep. Returns (in_maps, has_v0)."""
    x = np.asarray(x, dtype=np.float32)
    ln_gamma = np.asarray(ln_gamma, dtype=np.float32)
    ln_beta = np.asarray(ln_beta, dtype=np.float32)
    w_qkv = np.asarray(w_qkv, dtype=np.float32)
    w_out = np.asarray(w_out, dtype=np.float32)

    wsc = w_qkv.copy()
    wsc[:, :INNER] *= SCALE                      # fold attn scale into q
    wfold = ln_gamma[:, None] * wsc              # fold LN gamma
    u = wfold.sum(axis=0)                        # [3*INNER]
    v0 = ln_beta @ wsc                           # [3*INNER]
    has_v0 = bool(np.any(v0 != 0.0))

    wq, wk, wv_all = np.split(wfold, 3, axis=1)
    uq, uk, uv_all = np.split(u, 3)
    v0q, v0k, v0v_all = np.split(v0, 3)

    in_maps = []
    for c in range(8):
        b = c // 4
        hs = (c % 4) * HL * DH
        sl = slice(hs, hs + HL * DH)
        xb = x[b]                                           # [2048, 1024]
        wqk_loc = np.concatenate([wq[:, sl], wk[:, sl]], axis=1)  # [1024, 512]
        in_maps.append({
            "xt": np.ascontiguousarray(xb.T).astype(_BF16),
            "wqk": np.ascontiguousarray(wqk_loc).astype(_BF16),
            "wv": np.ascontiguousarray(wv_all[:, sl]).astype(_BF16),
            "wout": np.ascontiguousarray(w_out[sl, :]),
            "nuqk": np.concatenate([uq[sl], uk[sl]]).astype(_BF16),
            "nuv": uv_all[sl].astype(_BF16),
            "v0qk": np.concatenate([v0q[sl], v0k[sl]]).astype(np.float32),
            "v0v": v0v_all[sl].astype(np.float32),
        })
    return in_maps, has_v0


def run(x, ln_gamma, ln_beta, w_qkv, w_out, b_out, trace=False, trace_kwargs=None):
    in_maps, has_v0 = _prep_inputs(x, ln_gamma, ln_beta, w_qkv, w_out, b_out)
    key = ("nc", has_v0)
    if key not in _CACHE:
        _CACHE[key] = _build(has_v0)
    nc = _CACHE[key]
    kwargs = {}
    if trace:
        kwargs = dict(trace=True, trace_cores=[0],
                      stitch_traces=False, **(trace_kwargs or {}))
    res = bass_utils.run_bass_kernel_spmd(
        nc, in_maps, core_ids=list(range(8)), **kwargs)

    b_out = np.asarray(b_out, dtype=np.float32)
    out = np.zeros((B, N, DIM), dtype=np.float32)
    for b in range(B):
        acc = np.zeros((DIM, T), dtype=np.float32)
        for c in range(4 * b, 4 * b + 4):
            acc += res.results[c]["outp0"]
            acc += res.results[c]["outp1"]
        out[b] = acc.T + b_out
    return out, res


def kernel(x, ln_gamma, ln_beta, w_qkv, w_out, b_out):
    out, _ = run(x, ln_gamma, ln_beta, w_qkv, w_out, b_out, trace=False)
    return out



# revision 11
# speedup vs baseline: 1.0177x; 1.0177x over previous
"""Trainium2 Bass kernel for nn_Attention (LN -> QKV -> softmax attn -> out proj).

Sharding: 8 cores; core c handles batch b=c//4 and heads [4*(c%4), 4*(c%4)+4).
Each core emits one bf16 partial [DIM, T]; the host sums 4 partials per batch,
transposes, and adds b_out.

Device pipeline per core (fp16 matmuls, fp32 PSUM):
  A) One pass over x^T tiles with x as the PE stationary:
     v = x^T-tile @ wv_aug (wv with a ones rider column -> Sx lands in column
     layout for free); Sxx via tiny N=1 matmuls with Square(x) stationary.
     rsqrt(var+eps) via bit-trick + Newton on the DVE, all in SBUF column
     layout (no DMA transposes). LN mean/gamma folds applied algebraically:
     qk via rank-1 (-mu u) PSUM accumulation, v via DVE outer-product fixup.
  B) QK projection: stationary = folded weights, moving = x^T -> q,k
     transposed [dh, t] with both heads of a stack split across partition
     halves. r_q folded into q via broadcast multiply; r_k rides the exp's
     per-partition scale.
  C) Attention in head pairs (one stack at a time): the two heads' score
     matmuls run concurrently in disjoint PE row groups (K=64 row tiling);
     one 2048-wide Exp per (pair, qp, kt) covers both heads; optionally a
     512-chunk is offloaded to the DVE as a bit-trick fast exp. P@V with a
     ones column appended to v accumulates the denominator in the same PSUM
     tile; per-qp reciprocal via DVE HW divide, normalize fused on copy-out.
  D) Output projection: both stacks accumulated into one PSUM tile, written
     as a single bf16 partial.
"""

import contextlib

import numpy as np

import concourse.bass as bass
import concourse.tile as tile
from concourse import bacc, mybir
from concourse import bass_utils

# Problem constants (hardcoded per contract)
B, N, DIM = 2, 2048, 1024
H, DH = 16, 64
INNER = H * DH
LN_EPS = 1e-5
ATTN_EPS = 1e-8
SCALE = DH ** -0.5

# Per-core constants
P = 128
T = N                 # tokens per core (one batch)
TT = T // P           # 16 token tiles of 128
NT4 = T // 512        # 4 token slabs of 512
KD = DIM // P         # 8 contraction tiles
HL = 4                # local heads per core
CQK = 2 * HL * DH     # 512 (q cols + k cols)
CV = HL * DH          # 256 (v cols)
GQK = CQK // P        # 4 col groups of 128
KT = T // P           # 16 key tiles of 128

# DVE fast-exp offload: number of 512-wide chunks (of 4) per pair-iter
# computed on the DVE via the Schraudolph bit trick in fp16.
EXP_DVE_CHUNKS = 1
FE_A = 1.4426950408889634 * 1024.0          # log2(e) * 2^10 (fp16 mantissa)
FE_B = 15.0 * 1024.0 - 58.0                 # fp16 exp bias - sawtooth adjust

f32 = mybir.dt.float32
fp16 = mybir.dt.float16
bf16 = mybir.dt.bfloat16
i16 = mybir.dt.int16
i32 = mybir.dt.int32
FT = mybir.ActivationFunctionType
ALU = mybir.AluOpType

import ml_dtypes
_BF16 = np.dtype(ml_dtypes.bfloat16)

_CACHE = {}


def _build(has_v0):
    nc = bacc.Bacc("TRN2", target_bir_lowering=False, debug=False)

    xt_d = nc.dram_tensor("xt", [DIM, T], fp16, kind="ExternalInput").ap()
    wqk_d = nc.dram_tensor("wqk", [DIM, CQK], fp16, kind="ExternalInput").ap()
    wva_d = nc.dram_tensor("wva", [DIM, CV + 1], fp16, kind="ExternalInput").ap()
    wout_d = nc.dram_tensor("wout", [2 * P, DIM], fp16, kind="ExternalInput").ap()
    nuqk_d = nc.dram_tensor("nuqk", [CQK], fp16, kind="ExternalInput").ap()
    nuv_d = nc.dram_tensor("nuv", [CV], fp16, kind="ExternalInput").ap()
    v0qk_d = nc.dram_tensor("v0qk", [CQK], f32, kind="ExternalInput").ap()
    v0v_d = nc.dram_tensor("v0v", [CV], f32, kind="ExternalInput").ap()
    outp_d = nc.dram_tensor("outp", [DIM, T], bf16, kind="ExternalOutput").ap()

    with tile.TileContext(nc) as tc, contextlib.ExitStack() as ctx:
        pers = ctx.enter_context(tc.tile_pool(name="pers", bufs=1))
        dram = ctx.enter_context(tc.tile_pool(name="dram", bufs=1, space="DRAM"))

        qkT = pers.tile([P, GQK, T], fp16)           # q/k transposed, stacked
        vaug = pers.tile([P, KT, HL, DH + 1], fp16)  # v + ones column
        outT = pers.tile([P, 2, T], fp16)            # attention out (transposed)
        wout_sb = pers.tile([P, 2, DIM], fp16)
        r_c = pers.tile([P, TT], f32)                # rsqrt(var+eps), col layout
        rFE_c = pers.tile([P, TT], f32)              # r * FE_A (fast-exp scale)
        mur_c = pers.tile([P, TT], f32)              # mu * r, col layout
        nmu_row = pers.tile([1, T], fp16)            # -mu, row layout
        dbc = pers.tile([P, 2, T // 2], f32)         # 1/denom broadcast (per qp)

        nmu_dram = dram.tile([1, T], fp16)
        r_dram = dram.tile([1, T], fp16)
        dnm_dram = dram.tile([2, T // 2], f32)
        rdn_dram = dram.tile([2, T // 2], f32)

        nc.vector.memset(vaug[:], 1.0)

        # ---------------- Phase A+B: stats + QKV projection ----------------
        with tc.tile_pool(name="pab", bufs=1) as pab, \
             tc.tile_pool(name="pabd", bufs=2) as pabd, \
             tc.tile_pool(name="pgen", bufs=4, space="PSUM") as pgen, \
             tc.tile_pool(name="pgv", bufs=3, space="PSUM") as pgv, \
             tc.tile_pool(name="pgs", bufs=1, space="PSUM") as pgs:

            xt_sb = pab.tile([P, KD, T], fp16)
            for kt in range(KD):
                nc.sync.dma_start(xt_sb[:, kt], xt_d[kt * P:(kt + 1) * P, :])
            wqk_sb = pab.tile([P, KD, CQK], fp16)
            nc.sync.dma_start(wqk_sb[:], wqk_d.rearrange("(o p) c -> p o c", p=P))
            wva_sb = pab.tile([P, KD, CV + 1], fp16)
            nc.sync.dma_start(wva_sb[:], wva_d.rearrange("(o p) c -> p o c", p=P))
            uqkr_sb = pab.tile([1, CQK], fp16)
            nc.sync.dma_start(uqkr_sb[:], nuqk_d[None, :])
            uv_bc = pab.tile([P, CV], fp16)
            nc.sync.dma_start(uv_bc[:], nuv_d[None, :].to_broadcast([P, CV]))
            if has_v0:
                v0qk_sb = pab.tile([P, GQK], f32)
                nc.sync.dma_start(v0qk_sb[:], v0qk_d.rearrange("(g p) -> p g", p=P))
                v0v_bc = pab.tile([P, CV], f32)
                nc.sync.dma_start(v0v_bc[:], v0v_d[None, :].to_broadcast([P, CV]))

            r_bc = pab.tile([P, T], fp16)
            sxc = pab.tile([P, TT], f32)
            sxxc = pab.tile([P, TT], f32)
            mu_cc = pab.tile([P, TT], f32)
            magic = pab.tile([P, TT], i32)
            nc.vector.memset(magic[:], 0x5F3759DF)
            ex2e = pab.tile([P, TT], f32)
            mu2 = pab.tile([P, TT], f32)
            ve = pab.tile([P, TT], f32)
            y0i = pab.tile([P, TT], i32)
            t0 = pab.tile([P, TT], f32)
            nmu16_c = pab.tile([P, TT], fp16)
            r16_c = pab.tile([P, TT], fp16)
            vtmp = pab.tile([P, HL, DH], f32)
            ones_col = pab.tile([P, 1], fp16)
            nc.vector.memset(ones_col[:], 1.0)

            bones = pab.tile([1, 1], fp16)
            nc.vector.memset(bones[:], 1.0)
            brow = pab.tile([1, 64], fp16)
            nc.vector.memset(brow[:], 1.0)
            warm_ps = pgen.tile([P, 512], f32, tag="qk", name="warm0")
            for _ in range(150):
                nc.tensor.matmul(warm_ps[64:65, 0:64], bones[:], brow[:],
                                 start=True, stop=True)

            # --- pass 1: v projection (+Sx rider) and Sxx, per token tile ---
            sxx_ps = pgs.tile([P, TT], f32, tag="sxx")
            for tt in range(TT):
                tsl = slice(tt * P, (tt + 1) * P)
                xsq = pabd.tile([P, KD, P], fp16, tag="xsq", name=f"xsq{tt}")
                nc.scalar.activation(xsq[:], xt_sb[:, :, tsl], FT.Square)
                psv = pgv.tile([P, CV + 1], f32, tag="v", name=f"v{tt}")
                for kt in range(KD):
                    nc.tensor.matmul(psv[:], xt_sb[:, kt, tsl], wva_sb[:, kt],
                                     start=(kt == 0), stop=(kt == KD - 1))
                    nc.tensor.matmul(sxx_ps[:, tt:tt + 1], xsq[:, kt],
                                     ones_col[:],
                                     start=(kt == 0), stop=(kt == KD - 1))
                # v raw (unscaled) into vaug; Sx rider into column stats
                psv3 = psv[:, 0:CV].rearrange("p (h d) -> p h d", h=HL)
                nc.vector.tensor_copy(vaug[:, tt, :, 0:DH], psv3)
                nc.vector.tensor_copy(sxc[:, tt:tt + 1], psv[:, CV:CV + 1])
            nc.vector.tensor_copy(sxxc[:], sxx_ps[:])

            # --- rsqrt(var+eps) on the DVE, all-column layout ---
            nc.vector.tensor_scalar(ex2e[:], sxxc[:], 1.0 / DIM, LN_EPS,
                                    ALU.mult, ALU.add)
            nc.vector.tensor_scalar_mul(mu_cc[:], sxc[:], 1.0 / DIM)
            nc.vector.tensor_tensor(mu2[:], mu_cc[:], mu_cc[:], ALU.mult)
            nc.vector.scalar_tensor_tensor(ve[:], mu2[:], -1.0, ex2e[:],
                                           ALU.mult, ALU.add)
            nc.vector.tensor_scalar(y0i[:], ve[:].bitcast(i32), 1, None,
                                    ALU.arith_shift_right)
            nc.vector.tensor_tensor(y0i[:], magic[:], y0i[:], ALU.subtract)
            y = y0i.bitcast(f32)
            for _ in range(3):
                nc.vector.tensor_tensor(t0[:], y[:], y[:], ALU.mult)
                nc.vector.tensor_tensor(t0[:], t0[:], ve[:], ALU.mult)
                nc.vector.tensor_scalar(t0[:], t0[:], -0.5, 1.5,
                                        ALU.mult, ALU.add)
                nc.vector.tensor_tensor(y[:], y[:], t0[:], ALU.mult)
            nc.vector.tensor_copy(r_c[:], y[:])
            nc.vector.tensor_scalar_mul(rFE_c[:], r_c[:], FE_A)
            nc.vector.tensor_tensor(mur_c[:], mu_cc[:], r_c[:], ALU.mult)
            nc.vector.tensor_scalar_mul(nmu16_c[:], mu_cc[:], -1.0)
            nc.vector.tensor_copy(r16_c[:], r_c[:])

            # row layouts: -mu (rank-1 qk fix) and r broadcast (q scale)
            nc.sync.dma_start(nmu_dram[0, :].rearrange("(o p) -> p o", p=P),
                              nmu16_c[:])
            nc.sync.dma_start(nmu_row[:], nmu_dram[:])
            nc.sync.dma_start(r_dram[0, :].rearrange("(o p) -> p o", p=P),
                              r16_c[:])
            nc.sync.dma_start(r_bc[:], r_dram[0:1, :].to_broadcast([P, T]))

            # --- v fixup: vaug = r*(vraw - mu*uv) [+ v0] ---
            for tt in range(TT):
                nc.vector.tensor_scalar_mul(
                    vtmp.rearrange("p h d -> p (h d)"), uv_bc[:],
                    mur_c[:, tt:tt + 1])
                nc.vector.scalar_tensor_tensor(
                    vaug[:, tt, :, 0:DH], vaug[:, tt, :, 0:DH],
                    r_c[:, tt:tt + 1], vtmp[:], ALU.mult, ALU.subtract)
                if has_v0:
                    v03 = v0v_bc.rearrange("p (h d) -> p h d", h=HL)
                    nc.vector.tensor_tensor(vaug[:, tt, :, 0:DH],
                                            vaug[:, tt, :, 0:DH], v03, ALU.add)

            # --- pass 2: qk projection; k groups first (no r_bc dep) ---
            for g in (2, 3, 0, 1):
                for t4 in range(NT4):
                    tsl = slice(t4 * 512, (t4 + 1) * 512)
                    ps = pgen.tile([P, 512], f32, tag="qk", name=f"qk{g}_{t4}")
                    for kt in range(KD):
                        nc.tensor.matmul(ps[:], wqk_sb[:, kt, g * P:(g + 1) * P],
                                         xt_sb[:, kt, tsl],
                                         start=(kt == 0), stop=False)
                    # rank-1 LayerNorm-mean correction: psum += u * (-mu)^T
                    nc.tensor.matmul(ps[:], uqkr_sb[0:1, g * P:(g + 1) * P],
                                     nmu_row[0:1, tsl], start=False, stop=True)
                    if g >= 2:
                        nc.scalar.copy(qkT[:, g, tsl], ps[:])
                    else:
                        nc.vector.tensor_tensor(qkT[:, g, tsl], ps[:],
                                                r_bc[:, tsl], ALU.mult)
                    if has_v0:
                        nc.vector.tensor_scalar_add(qkT[:, g, tsl],
                                                    qkT[:, g, tsl],
                                                    v0qk_sb[:, g:g + 1])

        # ---------------- Phase C: attention (head pairs) -------------------
        with tc.tile_pool(name="pat", bufs=2) as pat, \
             tc.tile_pool(name="pat1", bufs=1) as pat1, \
             tc.tile_pool(name="psc", bufs=1, space="PSUM") as psc, \
             tc.tile_pool(name="ppv", bufs=1, space="PSUM") as ppv:

            nc.sync.dma_start(wout_sb[:], wout_d.rearrange("(o p) c -> p o c", p=P))
            dn_row = [pat1.tile([1, T // 2], f32, name=f"dnr{i}")
                      for i in range(2)]
            dn2 = pat1.tile([P, 2, TT // 2], f32)
            rdn2 = pat1.tile([P, 2, TT // 2], f32)

            for hp in range(2):
                for qp in range(2):
                    qsl = slice(qp * 1024, (qp + 1) * 1024)
                    ps_o = [ppv.tile([DH + 1, 2, 512], f32, tag=f"pv{i}",
                                     name=f"pv{hp}_{qp}_{i}") for i in range(2)]
                    for kt in range(KT):
                        ksl = slice(kt * P, (kt + 1) * P)
                        ps_s = psc.tile([P, 4, 512], f32, tag="sc",
                                        name=f"sc{hp}_{qp}_{kt}")
                        for half in range(2):      # head row-halves, row-tiled
                            rw = slice(64 * half, 64 * half + 64)
                            for sub in range(2):
                                qs = slice(qp * 1024 + sub * 512,
                                           qp * 1024 + (sub + 1) * 512)
                                nc.tensor.matmul(
                                    ps_s[:, 2 * half + sub],
                                    qkT[rw, 2 + hp, ksl], qkT[rw, hp, qs],
                                    start=True, stop=True)
                        et = pat.tile([P, 4, 512], fp16, tag="exp",
                                      name=f"et{hp}_{qp}_{kt}")
                        nsc = 4 - EXP_DVE_CHUNKS
                        nc.scalar.activation(et[:, 0:nsc], ps_s[:, 0:nsc],
                                             FT.Exp, scale=r_c[:, kt:kt + 1])
                        if EXP_DVE_CHUNKS:
                            eti = et.bitcast(i16)
                            nc.vector.tensor_scalar(
                                eti[:, nsc:4], ps_s[:, nsc:4],
                                rFE_c[:, kt:kt + 1], FE_B, ALU.mult, ALU.add)
                        for half in range(2):
                            for sub in range(2):
                                nc.tensor.matmul(
                                    ps_o[half][:, sub],
                                    vaug[:, kt, 2 * hp + half, :],
                                    et[:, 2 * half + sub],
                                    start=(kt == 0), stop=(kt == KT - 1))
                    # copy out (unnormalized) + denominator rows
                    for half in range(2):
                        rw = slice(64 * half, 64 * half + 64)
                        nc.vector.tensor_copy(
                            outT[rw, hp, qsl],
                            ps_o[half][0:DH].rearrange("p s f -> p (s f)"))
                        nc.vector.tensor_copy(
                            dn_row[half][:],
                            ps_o[half][DH:DH + 1].rearrange("p s f -> p (s f)"))
                        nc.sync.dma_start(dnm_dram[half:half + 1, :],
                                          dn_row[half][:])
                    # reciprocal in column layout, broadcast back
                    nc.sync.dma_start(
                        dn2[:], dnm_dram.rearrange("h (p o) -> p h o", p=P))
                    nc.vector.reciprocal(rdn2[:], dn2[:])
                    nc.sync.dma_start(
                        rdn_dram.rearrange("h (p o) -> p h o", p=P), rdn2[:])
                    for half in range(2):
                        rw = slice(64 * half, 64 * half + 64)
                        nc.sync.dma_start(
                            dbc[rw, qp, :],
                            rdn_dram[half:half + 1, :].to_broadcast([64, 1024]))
                        nc.vector.tensor_tensor(outT[rw, hp, qsl],
                                                outT[rw, hp, qsl],
                                                dbc[rw, qp, :], ALU.mult)

        # ---------------- Phase D: output projection ------------------------
        with tc.tile_pool(name="pdo", bufs=3) as pdo, \
             tc.tile_pool(name="pop", bufs=4, space="PSUM") as pop:
            for oc in range(DIM // P):
                for t4 in range(NT4):
                    tsl = slice(t4 * 512, (t4 + 1) * 512)
                    ps = pop.tile([P, 512], f32, tag="op", name=f"op{oc}_{t4}")
                    nc.tensor.matmul(ps[:], wout_sb[:, 0, oc * P:(oc + 1) * P],
                                     outT[:, 0, tsl], start=True, stop=False)
                    nc.tensor.matmul(ps[:], wout_sb[:, 1, oc * P:(oc + 1) * P],
                                     outT[:, 1, tsl], start=False, stop=True)
                    osb = pdo.tile([P, 512], bf16, tag="osb")
                    if (oc + t4) % 2 == 0:
                        nc.vector.tensor_copy(osb[:], ps[:])
                    else:
                        nc.scalar.copy(osb[:], ps[:])
                    nc.sync.dma_start(outp_d[oc * P:(oc + 1) * P, tsl], osb[:])

    nc.compile()
    return nc


def _prep_inputs(x, ln_gamma, ln_beta, w_qkv, w_out, b_out):
    """Host-side sharding/layout prep. Returns (in_maps, has_v0)."""
    x = np.asarray(x, dtype=np.float32)
    ln_gamma = np.asarray(ln_gamma, dtype=np.float32)
    ln_beta = np.asarray(ln_beta, dtype=np.float32)
    w_qkv = np.asarray(w_qkv, dtype=np.float32)
    w_out = np.asarray(w_out, dtype=np.float32)

    wsc = w_qkv.copy()
    wsc[:, :INNER] *= SCALE                      # fold attn scale into q
    wfold = ln_gamma[:, None] * wsc              # fold LN gamma
    u = wfold.sum(axis=0)                        # [3*INNER]
    v0 = ln_beta @ wsc                           # [3*INNER]
    has_v0 = bool(np.any(v0 != 0.0))

    wq, wk, wv_all = np.split(wfold, 3, axis=1)
    uq, uk, uv_all = np.split(u, 3)
    v0q, v0k, v0v_all = np.split(v0, 3)

    in_maps = []
    for c in range(8):
        b = c // 4
        hs = (c % 4) * HL * DH
        sl = slice(hs, hs + HL * DH)
        xb = x[b]                                           # [2048, 1024]
        wqk_loc = np.concatenate([wq[:, sl], wk[:, sl]], axis=1)  # [1024, 512]
        wva_loc = np.concatenate(
            [wv_all[:, sl], np.ones((DIM, 1), np.float32)], axis=1)
        in_maps.append({
            "xt": np.ascontiguousarray(xb.T).astype(np.float16),
            "wqk": np.ascontiguousarray(wqk_loc).astype(np.float16),
            "wva": np.ascontiguousarray(wva_loc).astype(np.float16),
            "wout": np.ascontiguousarray(w_out[sl, :]).astype(np.float16),
            "nuqk": np.concatenate([uq[sl], uk[sl]]).astype(np.float16),
            "nuv": uv_all[sl].astype(np.float16),
            "v0qk": np.concatenate([v0q[sl], v0k[sl]]).astype(np.float32),
            "v0v": v0v_all[sl].astype(np.float32),
        })
    return in_maps, has_v0


def run(x, ln_gamma, ln_beta, w_qkv, w_out, b_out, trace=False, trace_kwargs=None):
    in_maps, has_v0 = _prep_inputs(x, ln_gamma, ln_beta, w_qkv, w_out, b_out)
    key = ("nc", has_v0)
    if key not in _CACHE:
        _CACHE[key] = _build(has_v0)
    nc = _CACHE[key]
    kwargs = {}
    if trace:
        kwargs = dict(trace=True, trace_cores=[0],
                      stitch_traces=False, **(trace_kwargs or {}))
    res = bass_utils.run_bass_kernel_spmd(
        nc, in_maps, core_ids=list(range(8)), **kwargs)

    b_out = np.asarray(b_out, dtype=np.float32)
    out = np.zeros((B, N, DIM), dtype=np.float32)
    for b in range(B):
        acc = np.zeros((DIM, T), dtype=np.float32)
        for c in range(4 * b, 4 * b + 4):
            acc += np.asarray(res.results[c]["outp"]).astype(np.float32)
        out[b] = acc.T + b_out
    return out, res


def kernel(x, ln_gamma, ln_beta, w_qkv, w_out, b_out):
    out, _ = run(x, ln_gamma, ln_beta, w_qkv, w_out, b_out, trace=False)
    return out
